# revision 9
# baseline (speedup 1.0000x reference)
"""Trainium2 Bass kernel for the 1-D Bessel (von Mises-like) kernel matrix:

    K[i, j] = I0(2a * cos(pi * (x_i - y_j))) * exp(-2a),   a = 10

Algorithm
---------
K depends on d = x_i - y_j only through the periodic even function
h(d) = I0(20 cos(pi d)) e^-20.  Its log has a rapidly converging Fourier
cosine series; truncated at KH=25 harmonics (trunc err 6.3e-4):

    log h(d) ~ b0 + sum_{k=1..25} b_k cos(2 pi k d)

and cos(2 pi k (x - y)) = cos(2pi k x) cos(2pi k y) + sin(2pi k x) sin(2pi k y),
so log K is a rank-51 bilinear form of trig feature matrices.  bf16 rounding
of the features is repaired with hi/lo cross-correction rows for the constant
+ top KS=18 harmonics (37 rows, contributing uh*vl + ul*vh), packed together
with the base rows into a SINGLE K=128 matmul operand (51 + 74 = 125 rows):

    v = log2(K * 2^16) / (16 * lam) = U.T @ V   (one bf16 matmul pass,
                                                 fp32 PSUM)

The 2^16 scale and the 1/(16*lam) normalization are folded into the U-side
coefficients (lam normalizes the quartic below to monic form).

PSUM is drained to fp16 out = K * 2^16 by TWO engines in parallel:
  * ACT: fused exp, activation scale = 16*lam*ln2  (most tiles)
  * DVE: two custom ops -- a 7-stage monic-quartic 2^v approximation
         w = (((v+a)v+b)v+c)v + 1, then 4 squarings w^16  (offloaded tiles)
Max pointwise rel err ~1.1e-3 (ACT) / ~1.6e-3 (DVE), L2 ~3e-4.

fp16 out spans the normal range (K*2^16 in [1.3e-4, 5.9e3]); the host
multiplies by the exact 2^-16 and upcasts.  Output DMA (16 MiB/core at
~358 GB/s ~ 47 us) is the roofline; engine streams (ACT ~46 us,
DVE ~40 us, PE ~33 us) hide under it.

The tiny [128 x 8192] trig features are precomputed on host in float64.
"""

import os
import sys

import numpy as np

sys.path.insert(0, "/opt/trn_rl_repo")

A = 10.0
NX = 8192
NY = 8192
N_CORES = 8
MX = NX // N_CORES  # 1024 rows of x per core
KH = 25   # harmonics kept; base rows = 1 + 2*25 = 51
KS = 18   # harmonics getting hi/lo cross correction (+ constant row): 37 rows

# quartic 2^u fit on u in [-0.804, 0.783] (u = log2(K*2^16)/16), p(0)=1:
#   p(u) = 1 + c1 u + c2 u^2 + c3 u^3 + c4 u^4, max rel err 3.0e-5.
# lam = c4^(-1/4) makes the quartic monic in v = u/lam:
#   p - 1 = (((v + QA)v + QB)v + QC)v
LAM = 3.20681833173408
QA = 1.862280190508648
QB = 2.471649549398278
QC = 2.2223961635302145
ACT_SCALE = 16.0 * LAM * 0.6931471805599453  # exp(v * ACT_SCALE) = K * 2^16
FEAT_S = 1.0 / ACT_SCALE                     # ln-units -> v-units

# Fourier cosine coefficients of log(I0(20 cos(pi d))) - 20 on d in [0, 1),
# computed offline in float64 via FFT of the exact series evaluation.
_B0 = -9.320623105523872
_BK = [
    7.970447139028089, -1.4358756600553582, 0.5530401566383198,
    -0.27432647869384885, 0.1547723650507224, -0.09433791302730635,
    0.060502068515108406, -0.04020530135648252, 0.027418113277826187,
    -0.01906554834357182, 0.013458315954332174, -0.009613552975863679,
    0.0069329638057468446, -0.005038947804517573, 0.003686131354141929,
    -0.00271122806102214, 0.00200343687917714, -0.0014863506699641636,
    0.00110656955440988, -0.0008263523699001975, 0.000618771677773785,
    -0.00046446052148687905, 0.00034939361165105417, -0.0002633536495551932,
    0.00019885898700602698,
]

# of the 64 [128,1024] psum tiles per core, these go to the DVE 2^v path
# instead of ACT exp (19 tiles, spread evenly: balances ACT ~1.02us vs
# DVE ~2.35us per tile).  Tiles 0-1 stay ACT so the first output chunk
# drains fast; the last tiles stay ACT to keep the kernel tail short.
DVE_TILES = frozenset(
    [2, 6, 9, 13, 16, 19, 23, 26, 29, 33, 36, 40, 43, 46, 50, 53, 56, 59, 61]
)

_NC_CACHE = None
_DVE_OPS = None
LAST_EXEC_TIME_NS = None
LAST_TRACE_PATH = None


def _register_dve_ops():
    """Register the two custom DVE ops (documented runtime extension path:
    append to dve_ops.OPS + the name->row map; sha computed locally)."""
    global _DVE_OPS
    if _DVE_OPS is not None:
        return _DVE_OPS

    from concourse.dve_ops import (
        CUSTOM_DVE_SPECS,
        OPS,
        _SUB_OPCODE_FOR_NAME,
        DveOp,
    )
    from concourse.dve_spec import C0, C1, C2, One, Spec, Src0
    from concourse.dve_spec import _has_src1 as has_src1
    from concourse.dve_spec import lower
    from concourse.dve_uop import DveOpSpec

    def mk(name, spec, subdim=False):
        if name in _SUB_OPCODE_FOR_NAME:
            op = next(o for o in OPS if o.name == name)
            return op
        row = max(_SUB_OPCODE_FOR_NAME.values()) + 1
        _SUB_OPCODE_FOR_NAME[name] = row
        shas = {}
        for ver in ("v3", "v4"):
            try:
                uops = lower(spec, ver=ver)
                shas[ver] = DveOpSpec(
                    name=name, opcode=row, uops=uops, rd1_en=has_src1(spec)
                ).sha(ver)
            except Exception:
                pass
        op = DveOp(name, spec, subdim=subdim, uops_sha=shas)
        OPS.append(op)
        CUSTOM_DVE_SPECS[name] = spec
        return op

    # w = (((v + C0) v + C1) v + C2) v + 1  ~=  2^(lam v) for the chosen
    # monic-normalized quartic constants (7 stages)
    v = Src0
    q = ((((v + C0) * v + C1) * v + C2) * v) + One
    exp2q = mk(
        "ANT_EXP2_MONIC_Q",
        Spec(
            body=q,
            reference=lambda in0, in1, s0, s1, imm2: (
                (((in0 + s0) * in0 + s1) * in0 + imm2) * in0 + 1.0
            ),
        ),
    )

    # w -> w^16 by 4 squarings (4 stages)
    w2 = Src0 * Src0
    w4 = w2 * w2
    w8 = w4 * w4
    w16 = w8 * w8
    pow16 = mk(
        "ANT_POW16",
        Spec(
            body=w16,
            reference=lambda in0, in1, s0, s1, imm2: (in0.astype(np.float32))
            ** 16,
        ),
    )

    _DVE_OPS = (exp2q, pow16)
    return _DVE_OPS


def _features(x, y):
    """Host-side float64 trig features -> packed bf16 matmul operands.

    U rows (x side, b_k and the v-normalization folded in):
      [0..50]    base hi rows: const, cos 1..KH, sin 1..KH  (bf16 hi)
      [51..87]   hi rows of the split set (const + cos/sin 1..KS)
      [88..124]  lo rows of the split set
      [125..127] zero
    V rows (y side):
      [0..50]    base hi rows: 1, cos, sin
      [51..87]   LO rows of the split set
      [88..124]  HI rows of the split set
      [125..127] zero
    so U.T @ V = sum_base uh*vh + sum_split (uh*vl + ul*vh).
    """
    import ml_dtypes

    bf16 = ml_dtypes.bfloat16

    xf = np.asarray(x, np.float32).reshape(-1).astype(np.float64)
    yf = np.asarray(y, np.float32).reshape(-1).astype(np.float64)
    ks = np.arange(1, KH + 1, dtype=np.float64)[:, None]
    bk = np.array(_BK, np.float64)[:, None] * FEAT_S

    ang_x = (2.0 * np.pi) * ks * xf[None, :]
    u = np.zeros((128, xf.size), np.float64)
    u[0] = (_B0 + 16.0 * 0.6931471805599453) * FEAT_S  # fold 2^16 fp16 scale
    u[1 : KH + 1] = bk * np.cos(ang_x)
    u[KH + 1 : 2 * KH + 1] = bk * np.sin(ang_x)

    ang_y = (2.0 * np.pi) * ks * yf[None, :]
    vv = np.zeros((128, yf.size), np.float64)
    vv[0] = 1.0
    vv[1 : KH + 1] = np.cos(ang_y)
    vv[KH + 1 : 2 * KH + 1] = np.sin(ang_y)

    nb = 2 * KH + 1  # 51 base rows
    split = np.r_[0 : KS + 1, KH + 1 : KH + 1 + KS]  # 37 rows
    ns = split.size

    uh = u.astype(bf16)
    ul = (u - uh.astype(np.float64)).astype(bf16)
    vh = vv.astype(bf16)
    vl = (vv - vh.astype(np.float64)).astype(bf16)

    up = np.zeros((128, xf.size), bf16)
    vp = np.zeros((128, yf.size), bf16)
    up[:nb] = uh[:nb]
    up[nb : nb + ns] = uh[split]
    up[nb + ns : nb + 2 * ns] = ul[split]
    vp[:nb] = vh[:nb]
    vp[nb : nb + ns] = vl[split]
    vp[nb + ns : nb + 2 * ns] = vh[split]
    return up, vp


def _build():
    """Build + compile the per-core Bass/Tile kernel (cached)."""
    global _NC_CACHE
    if _NC_CACHE is not None:
        return _NC_CACHE

    from concourse import bacc, mybir
    import concourse.tile as tile

    f32 = mybir.dt.float32
    f16 = mybir.dt.float16
    bf16 = mybir.dt.bfloat16

    if DVE_TILES:
        exp2q, pow16 = _register_dve_ops()

    nc = bacc.Bacc(
        "TRN2", target_bir_lowering=False, debug=False, num_devices=N_CORES
    )
    ux_d = nc.dram_tensor("ux", [128, MX], bf16, kind="ExternalInput").ap()
    vy_d = nc.dram_tensor("vy", [128, NY], bf16, kind="ExternalInput").ap()
    out_d = nc.dram_tensor("out", [MX, NY], f16, kind="ExternalOutput").ap()

    n_mt = MX // 128   # 8 row blocks
    n_ng = NY // 2048  # 4 vy col groups of 2048
    n_q = NY // 1024   # 8 psum-sized col quarters of 1024 per row block

    with tile.TileContext(nc) as tc:
        with (
            tc.tile_pool(name="wpool", bufs=1) as wpool,
            tc.tile_pool(name="vpool", bufs=n_ng) as vpool,
            tc.tile_pool(name="pspool", bufs=4, space="PSUM") as pspool,
            tc.tile_pool(name="dvepool", bufs=4) as dvepool,
            tc.tile_pool(name="opool", bufs=8) as opool,
        ):
            # input loads: the m=0 slice of ux plus the first 512 vy cols
            # land first so the first matmul can start ~0.5 us in; the rest
            # streams behind (each DMA issue costs ~0.65 us on the sync
            # sequencer, so keep the count modest)
            uxa_t = wpool.tile([128, 128], bf16, name="uxa_t", tag="uxa_t")
            ux_t = wpool.tile([128, MX], bf16, name="ux_t", tag="ux_t")
            vys = []
            for ng in range(n_ng):
                vy_t = vpool.tile([128, 2048], bf16, name=f"vy_{ng}", tag="vy")
                vys.append(vy_t)
            nc.sync.dma_start(uxa_t[:], ux_d[:, 0:128])
            nc.sync.dma_start(vys[0][:, 0:512], vy_d[:, 0:512])
            nc.sync.dma_start(vys[0][:, 512:1024], vy_d[:, 512:1024])
            nc.sync.dma_start(vys[0][:, 1024:2048], vy_d[:, 1024:2048])
            nc.sync.dma_start(vys[1][:], vy_d[:, 2048:4096])
            # the full ux is only needed from the m=1 row block (~10 us in),
            # so it loads behind the vy groups m=0 consumes first
            nc.sync.dma_start(ux_t[:], ux_d[:])

            # warm_t zeroed on the otherwise-idle Pool engine (the tile
            # framework requires a write before any read)
            warm_t = wpool.tile([128, 512], bf16, name="warm_t", tag="warm_t")
            nc.gpsimd.memset(warm_t[:], 0.0)

            # ACT warm-up: a tiny Exp right away pulls the ~1.3 us activation
            # table load into the head, off the first real tile's drain.
            warm_a = wpool.tile([128, 8], f16, name="warm_a", tag="warm_a")
            nc.scalar.activation(
                warm_a[:],
                warm_t[:, 0:8],
                mybir.ActivationFunctionType.Exp,
                scale=0.0,
            )

            # PE warm-up: dummy matmuls on the zeroed tile keep the PE busy
            # while inputs stream in, so the HAM clock gate is at 2.4 GHz
            # when the real matmuls start.
            warm_ps = pspool.tile([128, 512], f32, name="warm_ps", tag="ps")
            for _w in range(10):
                nc.tensor.matmul(
                    warm_ps[:, 0:512],
                    warm_t[:, 0:128],
                    warm_t[:],
                    start=True,
                    stop=True,
                )
            for ng in range(2, n_ng):
                sl = slice(ng * 2048, (ng + 1) * 2048)
                nc.sync.dma_start(vys[ng][:], vy_d[:, sl])

            for m in range(n_mt):
                msl = slice(m * 128, (m + 1) * 128)
                ux_src = uxa_t[:, 0:128] if m == 0 else ux_t[:, msl]
                for cg in range(n_q // 2):  # 2048-col output chunks
                    out_t = opool.tile(
                        [128, 2048], f16, name=f"out_{m}_{cg}", tag="out_t"
                    )
                    for sub in range(2):
                        q = cg * 2 + sub
                        osl = slice(sub * 1024, (sub + 1) * 1024)
                        ps = pspool.tile(
                            [128, 1024], f32, name=f"ps_{m}_{q}", tag="ps"
                        )
                        for s in range(2):
                            ssl = slice(s * 512, (s + 1) * 512)
                            nc.tensor.matmul(
                                ps[:, ssl],
                                ux_src,
                                vys[q // 2][:, (q % 2) * 1024 + s * 512 :
                                            (q % 2) * 1024 + (s + 1) * 512],
                                start=True, stop=True,
                            )
                        if (m * n_q + q) in DVE_TILES:
                            # DVE drain: monic quartic 2^v then w^16
                            w_t = dvepool.tile(
                                [128, 1024], f32, name=f"w_{m}_{q}", tag="w_t"
                            )
                            nc.vector._custom_dve(
                                exp2q, out=w_t[:], in0=ps[:],
                                s0=QA, s1=QB, imm2=QC,
                            )
                            nc.vector._custom_dve(
                                pow16, out=out_t[:, osl], in0=w_t[:],
                            )
                        else:
                            # ACT drain: fp16 out = exp(v * 16 lam ln2)
                            # = K * 2^16, always in fp16 normal range
                            nc.scalar.activation(
                                out_t[:, osl],
                                ps[:],
                                mybir.ActivationFunctionType.Exp,
                                scale=ACT_SCALE,
                            )
                    nc.sync.dma_start(
                        out_d[msl, cg * 2048 : (cg + 1) * 2048],
                        out_t[:],
                    )

    nc.compile()
    _NC_CACHE = nc
    return nc


def kernel(x: np.ndarray, y: np.ndarray) -> np.ndarray:
    global LAST_EXEC_TIME_NS, LAST_TRACE_PATH
    from concourse import bass_utils

    up, vp = _features(x, y)
    nc = _build()

    in_maps = [
        {"ux": np.ascontiguousarray(up[:, i * MX : (i + 1) * MX]), "vy": vp}
        for i in range(N_CORES)
    ]
    trace = bool(os.environ.get("BESSEL_TRACE"))
    res = bass_utils.run_bass_kernel_spmd(
        nc, in_maps, core_ids=list(range(N_CORES)), trace=trace
    )
    LAST_EXEC_TIME_NS = res.exec_time_ns
    if res.instructions_and_trace is not None:
        LAST_TRACE_PATH = res.instructions_and_trace[1]
    out = np.empty((NX, NY), np.float32)
    for i in range(N_CORES):
        blk = out[i * MX : (i + 1) * MX]
        np.multiply(
            res.results[i]["out"].astype(np.float32),
            np.float32(2.0**-16),
            out=blk,
        )
    return out


# revision 10
# speedup vs baseline: 1.0289x; 1.0289x over previous
"""Trainium2 Bass kernel for the 1-D Bessel (von Mises-like) kernel matrix:

    K[i, j] = I0(2a * cos(pi * (x_i - y_j))) * exp(-2a),   a = 10

Algorithm
---------
K depends on d = x_i - y_j only through the periodic even function
h(d) = I0(20 cos(pi d)) e^-20.  Its log has a rapidly converging Fourier
cosine series; truncated at KH=25 harmonics (trunc err 6.3e-4):

    log h(d) ~ b0 + sum_{k=1..25} b_k cos(2 pi k d)

and cos(2 pi k (x - y)) = cos(2pi k x) cos(2pi k y) + sin(2pi k x) sin(2pi k y),
so log K is a rank-51 bilinear form of trig feature matrices.  bf16 rounding
of the features is repaired with hi/lo cross-correction rows for the constant
+ top KS=18 harmonics (37 rows, contributing uh*vl + ul*vh), packed together
with the base rows into a SINGLE K=128 matmul operand (51 + 74 = 125 rows):

    v = log2(K * 2^16) / (16 * lam) = U.T @ V   (one bf16 matmul pass,
                                                 fp32 PSUM)

The 2^16 scale and the 1/(16*lam) normalization are folded into the U-side
coefficients (lam normalizes the quartic below to monic form).

PSUM is drained to fp16 out = K * 2^16 by TWO engines in parallel:
  * ACT: fused exp, activation scale = 16*lam*ln2  (most tiles)
  * DVE: two custom ops -- a 7-stage monic-quartic 2^v approximation
         w = (((v+a)v+b)v+c)v + 1, then 4 squarings w^16  (offloaded tiles)
Max pointwise rel err ~1.1e-3 (ACT) / ~1.6e-3 (DVE), L2 ~3e-4.

fp16 out spans the normal range (K*2^16 in [1.3e-4, 5.9e3]); the host
multiplies by the exact 2^-16 and upcasts.  Output DMA (16 MiB/core at
~358 GB/s ~ 47 us) is the roofline; engine streams (ACT ~46 us,
DVE ~40 us, PE ~33 us) hide under it.

The tiny [128 x 8192] trig features are precomputed on host in float64.
"""

import os
import sys

import numpy as np

sys.path.insert(0, "/opt/trn_rl_repo")

A = 10.0
NX = 8192
NY = 8192
N_CORES = 8
MX = NX // N_CORES  # 1024 rows of x per core
KH = 25   # harmonics kept; base rows = 1 + 2*25 = 51
KS = 18   # harmonics getting hi/lo cross correction (+ constant row): 37 rows

# quartic 2^u fit on u in [-0.804, 0.783] (u = log2(K*2^16)/16), p(0)=1:
#   p(u) = 1 + c1 u + c2 u^2 + c3 u^3 + c4 u^4, max rel err 3.0e-5.
# lam = c4^(-1/4) makes the quartic monic in v = u/lam:
#   p - 1 = (((v + QA)v + QB)v + QC)v
LAM = 3.20681833173408
QA = 1.862280190508648
QB = 2.471649549398278
QC = 2.2223961635302145
ACT_SCALE = 16.0 * LAM * 0.6931471805599453  # exp(v * ACT_SCALE) = K * 2^16
FEAT_S = 1.0 / ACT_SCALE                     # ln-units -> v-units

# Fourier cosine coefficients of log(I0(20 cos(pi d))) - 20 on d in [0, 1),
# computed offline in float64 via FFT of the exact series evaluation.
_B0 = -9.320623105523872
_BK = [
    7.970447139028089, -1.4358756600553582, 0.5530401566383198,
    -0.27432647869384885, 0.1547723650507224, -0.09433791302730635,
    0.060502068515108406, -0.04020530135648252, 0.027418113277826187,
    -0.01906554834357182, 0.013458315954332174, -0.009613552975863679,
    0.0069329638057468446, -0.005038947804517573, 0.003686131354141929,
    -0.00271122806102214, 0.00200343687917714, -0.0014863506699641636,
    0.00110656955440988, -0.0008263523699001975, 0.000618771677773785,
    -0.00046446052148687905, 0.00034939361165105417, -0.0002633536495551932,
    0.00019885898700602698,
]

# of the 64 [128,1024] psum tiles per core, these go to the DVE 2^v path
# instead of ACT exp (19 tiles, spread evenly: balances ACT ~1.02us vs
# DVE ~2.35us per tile).  All DVE tiles sit at EVEN positions — the first
# slot of their 2048-col output chunk — so the DVE op2 overlaps the
# partner ACT drain and every chunk completes on the ACT cadence (out-DMA
# descriptors drain in chunk order across all queues, so one late chunk
# stalls the whole store stream).  Tiles 0-1 and 62-63 stay ACT to keep
# the kernel head and tail short.
DVE_TILES = frozenset(
    [2, 6, 8, 12, 14, 18, 22, 24, 28, 30, 34, 38, 40, 44, 48, 50, 54, 56, 60]
)

_NC_CACHE = None
_DVE_OPS = None
LAST_EXEC_TIME_NS = None
LAST_TRACE_PATH = None


def _register_dve_ops():
    """Register the two custom DVE ops (documented runtime extension path:
    append to dve_ops.OPS + the name->row map; sha computed locally)."""
    global _DVE_OPS
    if _DVE_OPS is not None:
        return _DVE_OPS

    from concourse.dve_ops import (
        CUSTOM_DVE_SPECS,
        OPS,
        _SUB_OPCODE_FOR_NAME,
        DveOp,
    )
    from concourse.dve_spec import C0, C1, C2, One, Spec, Src0
    from concourse.dve_spec import _has_src1 as has_src1
    from concourse.dve_spec import lower
    from concourse.dve_uop import DveOpSpec

    def mk(name, spec, subdim=False):
        if name in _SUB_OPCODE_FOR_NAME:
            op = next(o for o in OPS if o.name == name)
            return op
        row = max(_SUB_OPCODE_FOR_NAME.values()) + 1
        _SUB_OPCODE_FOR_NAME[name] = row
        shas = {}
        for ver in ("v3", "v4"):
            try:
                uops = lower(spec, ver=ver)
                shas[ver] = DveOpSpec(
                    name=name, opcode=row, uops=uops, rd1_en=has_src1(spec)
                ).sha(ver)
            except Exception:
                pass
        op = DveOp(name, spec, subdim=subdim, uops_sha=shas)
        OPS.append(op)
        CUSTOM_DVE_SPECS[name] = spec
        return op

    # w = (((v + C0) v + C1) v + C2) v + 1  ~=  2^(lam v) for the chosen
    # monic-normalized quartic constants (7 stages)
    v = Src0
    q = ((((v + C0) * v + C1) * v + C2) * v) + One
    exp2q = mk(
        "ANT_EXP2_MONIC_Q",
        Spec(
            body=q,
            reference=lambda in0, in1, s0, s1, imm2: (
                (((in0 + s0) * in0 + s1) * in0 + imm2) * in0 + 1.0
            ),
        ),
    )

    # w -> w^16 by 4 squarings (4 stages)
    w2 = Src0 * Src0
    w4 = w2 * w2
    w8 = w4 * w4
    w16 = w8 * w8
    pow16 = mk(
        "ANT_POW16",
        Spec(
            body=w16,
            reference=lambda in0, in1, s0, s1, imm2: (in0.astype(np.float32))
            ** 16,
        ),
    )

    _DVE_OPS = (exp2q, pow16)
    return _DVE_OPS


def _features(x, y):
    """Host-side float64 trig features -> packed bf16 matmul operands.

    U rows (x side, b_k and the v-normalization folded in):
      [0..50]    base hi rows: const, cos 1..KH, sin 1..KH  (bf16 hi)
      [51..87]   hi rows of the split set (const + cos/sin 1..KS)
      [88..124]  lo rows of the split set
      [125..127] zero
    V rows (y side):
      [0..50]    base hi rows: 1, cos, sin
      [51..87]   LO rows of the split set
      [88..124]  HI rows of the split set
      [125..127] zero
    so U.T @ V = sum_base uh*vh + sum_split (uh*vl + ul*vh).
    """
    import ml_dtypes

    bf16 = ml_dtypes.bfloat16

    xf = np.asarray(x, np.float32).reshape(-1).astype(np.float64)
    yf = np.asarray(y, np.float32).reshape(-1).astype(np.float64)
    ks = np.arange(1, KH + 1, dtype=np.float64)[:, None]
    bk = np.array(_BK, np.float64)[:, None] * FEAT_S

    ang_x = (2.0 * np.pi) * ks * xf[None, :]
    u = np.zeros((128, xf.size), np.float64)
    u[0] = (_B0 + 16.0 * 0.6931471805599453) * FEAT_S  # fold 2^16 fp16 scale
    u[1 : KH + 1] = bk * np.cos(ang_x)
    u[KH + 1 : 2 * KH + 1] = bk * np.sin(ang_x)

    ang_y = (2.0 * np.pi) * ks * yf[None, :]
    vv = np.zeros((128, yf.size), np.float64)
    vv[0] = 1.0
    vv[1 : KH + 1] = np.cos(ang_y)
    vv[KH + 1 : 2 * KH + 1] = np.sin(ang_y)

    nb = 2 * KH + 1  # 51 base rows
    split = np.r_[0 : KS + 1, KH + 1 : KH + 1 + KS]  # 37 rows
    ns = split.size

    uh = u.astype(bf16)
    ul = (u - uh.astype(np.float64)).astype(bf16)
    vh = vv.astype(bf16)
    vl = (vv - vh.astype(np.float64)).astype(bf16)

    up = np.zeros((128, xf.size), bf16)
    vp = np.zeros((128, yf.size), bf16)
    up[:nb] = uh[:nb]
    up[nb : nb + ns] = uh[split]
    up[nb + ns : nb + 2 * ns] = ul[split]
    vp[:nb] = vh[:nb]
    vp[nb : nb + ns] = vl[split]
    vp[nb + ns : nb + 2 * ns] = vh[split]
    return up, vp


def _build():
    """Build + compile the per-core Bass/Tile kernel (cached)."""
    global _NC_CACHE
    if _NC_CACHE is not None:
        return _NC_CACHE

    from concourse import bacc, mybir
    import concourse.tile as tile

    f32 = mybir.dt.float32
    f16 = mybir.dt.float16
    bf16 = mybir.dt.bfloat16

    if DVE_TILES:
        exp2q, pow16 = _register_dve_ops()

    nc = bacc.Bacc(
        "TRN2", target_bir_lowering=False, debug=False, num_devices=N_CORES
    )
    ux_d = nc.dram_tensor("ux", [128, MX], bf16, kind="ExternalInput").ap()
    vy_d = nc.dram_tensor("vy", [128, NY], bf16, kind="ExternalInput").ap()
    out_d = nc.dram_tensor("out", [MX, NY], f16, kind="ExternalOutput").ap()

    n_mt = MX // 128   # 8 row blocks
    n_ng = NY // 2048  # 4 vy col groups of 2048
    n_q = NY // 1024   # 8 psum-sized col quarters of 1024 per row block

    with tile.TileContext(nc) as tc:
        with (
            tc.tile_pool(name="wpool", bufs=1) as wpool,
            tc.tile_pool(name="vpool", bufs=n_ng) as vpool,
            tc.tile_pool(name="pspool", bufs=4, space="PSUM") as pspool,
            tc.tile_pool(name="dvepool", bufs=4) as dvepool,
            tc.tile_pool(name="opool", bufs=8) as opool,
        ):
            # input loads: the m=0 slice of ux plus the first 512 vy cols
            # land first so the first matmul can start ~0.5 us in; the rest
            # streams behind (each DMA issue costs ~0.65 us on the sync
            # sequencer, so keep the count modest)
            uxa_t = wpool.tile([128, 128], bf16, name="uxa_t", tag="uxa_t")
            ux_t = wpool.tile([128, MX], bf16, name="ux_t", tag="ux_t")
            vys = []
            for ng in range(n_ng):
                vy_t = vpool.tile([128, 2048], bf16, name=f"vy_{ng}", tag="vy")
                vys.append(vy_t)
            nc.sync.dma_start(uxa_t[:], ux_d[:, 0:128])
            nc.sync.dma_start(vys[0][:, 0:512], vy_d[:, 0:512])
            nc.sync.dma_start(vys[0][:, 512:1024], vy_d[:, 512:1024])
            nc.sync.dma_start(vys[0][:, 1024:2048], vy_d[:, 1024:2048])
            nc.sync.dma_start(vys[1][:], vy_d[:, 2048:4096])
            # the full ux is only needed from the m=1 row block (~10 us in),
            # so it loads behind the vy groups m=0 consumes first
            nc.sync.dma_start(ux_t[:], ux_d[:])

            # warm_t zeroed on the otherwise-idle Pool engine (the tile
            # framework requires a write before any read)
            warm_t = wpool.tile([128, 512], bf16, name="warm_t", tag="warm_t")
            nc.gpsimd.memset(warm_t[:], 0.0)

            # ACT warm-up: a tiny Exp right away pulls the ~1.3 us activation
            # table load into the head, off the first real tile's drain.
            warm_a = wpool.tile([128, 8], f16, name="warm_a", tag="warm_a")
            nc.scalar.activation(
                warm_a[:],
                warm_t[:, 0:8],
                mybir.ActivationFunctionType.Exp,
                scale=0.0,
            )

            # PE warm-up: dummy matmuls on the zeroed tile keep the PE busy
            # while inputs stream in, so the HAM clock gate is at 2.4 GHz
            # when the real matmuls start.
            warm_ps = pspool.tile([128, 512], f32, name="warm_ps", tag="ps")
            for _w in range(10):
                nc.tensor.matmul(
                    warm_ps[:, 0:512],
                    warm_t[:, 0:128],
                    warm_t[:],
                    start=True,
                    stop=True,
                )
            for ng in range(2, n_ng):
                sl = slice(ng * 2048, (ng + 1) * 2048)
                nc.sync.dma_start(vys[ng][:], vy_d[:, sl])

            for m in range(n_mt):
                msl = slice(m * 128, (m + 1) * 128)
                ux_src = uxa_t[:, 0:128] if m == 0 else ux_t[:, msl]
                for cg in range(n_q // 2):  # 2048-col output chunks
                    out_t = opool.tile(
                        [128, 2048], f16, name=f"out_{m}_{cg}", tag="out_t"
                    )
                    for sub in range(2):
                        q = cg * 2 + sub
                        osl = slice(sub * 1024, (sub + 1) * 1024)
                        ps = pspool.tile(
                            [128, 1024], f32, name=f"ps_{m}_{q}", tag="ps"
                        )
                        for s in range(2):
                            ssl = slice(s * 512, (s + 1) * 512)
                            nc.tensor.matmul(
                                ps[:, ssl],
                                ux_src,
                                vys[q // 2][:, (q % 2) * 1024 + s * 512 :
                                            (q % 2) * 1024 + (s + 1) * 512],
                                start=True, stop=True,
                            )
                        if (m * n_q + q) in DVE_TILES:
                            # DVE drain: monic quartic 2^v then w^16
                            w_t = dvepool.tile(
                                [128, 1024], f32, name=f"w_{m}_{q}", tag="w_t"
                            )
                            nc.vector._custom_dve(
                                exp2q, out=w_t[:], in0=ps[:],
                                s0=QA, s1=QB, imm2=QC,
                            )
                            nc.vector._custom_dve(
                                pow16, out=out_t[:, osl], in0=w_t[:],
                            )
                        else:
                            # ACT drain: fp16 out = exp(v * 16 lam ln2)
                            # = K * 2^16, always in fp16 normal range
                            nc.scalar.activation(
                                out_t[:, osl],
                                ps[:],
                                mybir.ActivationFunctionType.Exp,
                                scale=ACT_SCALE,
                            )
                    nc.sync.dma_start(
                        out_d[msl, cg * 2048 : (cg + 1) * 2048],
                        out_t[:],
                    )

    nc.compile()
    _NC_CACHE = nc
    return nc


def kernel(x: np.ndarray, y: np.ndarray) -> np.ndarray:
    global LAST_EXEC_TIME_NS, LAST_TRACE_PATH
    from concourse import bass_utils

    up, vp = _features(x, y)
    nc = _build()

    in_maps = [
        {"ux": np.ascontiguousarray(up[:, i * MX : (i + 1) * MX]), "vy": vp}
        for i in range(N_CORES)
    ]
    trace = bool(os.environ.get("BESSEL_TRACE"))
    res = bass_utils.run_bass_kernel_spmd(
        nc, in_maps, core_ids=list(range(N_CORES)), trace=trace
    )
    LAST_EXEC_TIME_NS = res.exec_time_ns
    if res.instructions_and_trace is not None:
        LAST_TRACE_PATH = res.instructions_and_trace[1]
    out = np.empty((NX, NY), np.float32)
    for i in range(N_CORES):
        blk = out[i * MX : (i + 1) * MX]
        np.multiply(
            res.results[i]["out"].astype(np.float32),
            np.float32(2.0**-16),
            out=blk,
        )
    return out


# revision 12
# speedup vs baseline: 1.0516x; 1.0221x over previous
"""Trainium2 Bass kernel for the 1-D Bessel (von Mises-like) kernel matrix:

    K[i, j] = I0(2a * cos(pi * (x_i - y_j))) * exp(-2a),   a = 10

Algorithm
---------
K depends on d = x_i - y_j only through the periodic even function
h(d) = I0(20 cos(pi d)) e^-20.  Its log has a rapidly converging Fourier
cosine series; truncated at KH=25 harmonics (trunc err 6.3e-4):

    log h(d) ~ b0 + sum_{k=1..25} b_k cos(2 pi k d)

and cos(2 pi k (x - y)) = cos(2pi k x) cos(2pi k y) + sin(2pi k x) sin(2pi k y),
so log K is a rank-51 bilinear form of trig feature matrices.  bf16 rounding
of the features is repaired with hi/lo cross-correction rows for the constant
+ top KS=18 harmonics (37 rows, contributing uh*vl + ul*vh), packed together
with the base rows into a SINGLE K=128 matmul operand (51 + 74 = 125 rows):

    v = log2(K * 2^16) / (16 * lam) = U.T @ V   (one bf16 matmul pass,
                                                 fp32 PSUM)

The 2^16 scale and the 1/(16*lam) normalization are folded into the U-side
coefficients (lam normalizes the quartic below to monic form).

PSUM is drained to fp16 out = K * 2^16 by TWO engines in parallel:
  * ACT: fused exp, activation scale = 16*lam*ln2  (most tiles)
  * DVE: two custom ops -- a 7-stage monic-quartic 2^v approximation
         w = (((v+a)v+b)v+c)v + 1, then 4 squarings w^16  (offloaded tiles)
Max pointwise rel err ~1.1e-3 (ACT) / ~1.6e-3 (DVE), L2 ~3e-4.

fp16 out spans the normal range (K*2^16 in [1.3e-4, 5.9e3]); the host
multiplies by the exact 2^-16 and upcasts.  Output DMA (16 MiB/core at
~358 GB/s ~ 47 us) is the roofline; engine streams (ACT ~46 us,
DVE ~40 us, PE ~33 us) hide under it.

The tiny [128 x 8192] trig features are precomputed on host in float64.
"""

import os
import sys

import numpy as np

sys.path.insert(0, "/opt/trn_rl_repo")

A = 10.0
NX = 8192
NY = 8192
N_CORES = 8
MX = NX // N_CORES  # 1024 rows of x per core
KH = 25   # harmonics kept; base rows = 1 + 2*25 = 51
KS = 18   # harmonics getting hi/lo cross correction (+ constant row): 37 rows

# quartic 2^u fit on u in [-0.804, 0.783] (u = log2(K*2^16)/16), p(0)=1:
#   p(u) = 1 + c1 u + c2 u^2 + c3 u^3 + c4 u^4, max rel err 3.0e-5.
# lam = c4^(-1/4) makes the quartic monic in v = u/lam:
#   p - 1 = (((v + QA)v + QB)v + QC)v
LAM = 3.20681833173408
QA = 1.862280190508648
QB = 2.471649549398278
QC = 2.2223961635302145
ACT_SCALE = 16.0 * LAM * 0.6931471805599453  # exp(v * ACT_SCALE) = K * 2^16
FEAT_S = 1.0 / ACT_SCALE                     # ln-units -> v-units

# Fourier cosine coefficients of log(I0(20 cos(pi d))) - 20 on d in [0, 1),
# computed offline in float64 via FFT of the exact series evaluation.
_B0 = -9.320623105523872
_BK = [
    7.970447139028089, -1.4358756600553582, 0.5530401566383198,
    -0.27432647869384885, 0.1547723650507224, -0.09433791302730635,
    0.060502068515108406, -0.04020530135648252, 0.027418113277826187,
    -0.01906554834357182, 0.013458315954332174, -0.009613552975863679,
    0.0069329638057468446, -0.005038947804517573, 0.003686131354141929,
    -0.00271122806102214, 0.00200343687917714, -0.0014863506699641636,
    0.00110656955440988, -0.0008263523699001975, 0.000618771677773785,
    -0.00046446052148687905, 0.00034939361165105417, -0.0002633536495551932,
    0.00019885898700602698,
]

# of the 64 [128,1024] psum tiles per core, these go to the DVE 2^v path
# instead of ACT exp (19 tiles, spread evenly: balances ACT ~1.02us vs
# DVE ~2.35us per tile).  All DVE tiles sit at EVEN positions — the first
# slot of their 2048-col output chunk — so the DVE op2 overlaps the
# partner ACT drain and every chunk completes on the ACT cadence (out-DMA
# descriptors drain in chunk order across all queues, so one late chunk
# stalls the whole store stream).  Tiles 0-1 and 62-63 stay ACT to keep
# the kernel head and tail short.
DVE_TILES = frozenset(
    [6, 8, 12, 14, 18, 20, 24, 26, 30, 32, 36, 38, 42, 44, 48, 50, 54, 56]
)

_NC_CACHE = None
_DVE_OPS = None
LAST_EXEC_TIME_NS = None
LAST_TRACE_PATH = None


def _register_dve_ops():
    """Register the two custom DVE ops (documented runtime extension path:
    append to dve_ops.OPS + the name->row map; sha computed locally)."""
    global _DVE_OPS
    if _DVE_OPS is not None:
        return _DVE_OPS

    from concourse.dve_ops import (
        CUSTOM_DVE_SPECS,
        OPS,
        _SUB_OPCODE_FOR_NAME,
        DveOp,
    )
    from concourse.dve_spec import C0, C1, C2, One, Spec, Src0
    from concourse.dve_spec import _has_src1 as has_src1
    from concourse.dve_spec import lower
    from concourse.dve_uop import DveOpSpec

    def mk(name, spec, subdim=False):
        if name in _SUB_OPCODE_FOR_NAME:
            op = next(o for o in OPS if o.name == name)
            return op
        row = max(_SUB_OPCODE_FOR_NAME.values()) + 1
        _SUB_OPCODE_FOR_NAME[name] = row
        shas = {}
        for ver in ("v3", "v4"):
            try:
                uops = lower(spec, ver=ver)
                shas[ver] = DveOpSpec(
                    name=name, opcode=row, uops=uops, rd1_en=has_src1(spec)
                ).sha(ver)
            except Exception:
                pass
        op = DveOp(name, spec, subdim=subdim, uops_sha=shas)
        OPS.append(op)
        CUSTOM_DVE_SPECS[name] = spec
        return op

    # w = (((v + C0) v + C1) v + C2) v + 1  ~=  2^(lam v) for the chosen
    # monic-normalized quartic constants (7 stages)
    v = Src0
    q = ((((v + C0) * v + C1) * v + C2) * v) + One
    exp2q = mk(
        "ANT_EXP2_MONIC_Q",
        Spec(
            body=q,
            reference=lambda in0, in1, s0, s1, imm2: (
                (((in0 + s0) * in0 + s1) * in0 + imm2) * in0 + 1.0
            ),
        ),
    )

    # w -> w^16 by 4 squarings (4 stages)
    w2 = Src0 * Src0
    w4 = w2 * w2
    w8 = w4 * w4
    w16 = w8 * w8
    pow16 = mk(
        "ANT_POW16",
        Spec(
            body=w16,
            reference=lambda in0, in1, s0, s1, imm2: (in0.astype(np.float32))
            ** 16,
        ),
    )

    _DVE_OPS = (exp2q, pow16)
    return _DVE_OPS


def _features(x, y):
    """Host-side float64 trig features -> packed bf16 matmul operands.

    U rows (x side, b_k and the v-normalization folded in):
      [0..50]    base hi rows: const, cos 1..KH, sin 1..KH  (bf16 hi)
      [51..87]   hi rows of the split set (const + cos/sin 1..KS)
      [88..124]  lo rows of the split set
      [125..127] zero
    V rows (y side):
      [0..50]    base hi rows: 1, cos, sin
      [51..87]   LO rows of the split set
      [88..124]  HI rows of the split set
      [125..127] zero
    so U.T @ V = sum_base uh*vh + sum_split (uh*vl + ul*vh).
    """
    import ml_dtypes

    bf16 = ml_dtypes.bfloat16

    xf = np.asarray(x, np.float32).reshape(-1).astype(np.float64)
    yf = np.asarray(y, np.float32).reshape(-1).astype(np.float64)
    ks = np.arange(1, KH + 1, dtype=np.float64)[:, None]
    bk = np.array(_BK, np.float64)[:, None] * FEAT_S

    ang_x = (2.0 * np.pi) * ks * xf[None, :]
    u = np.zeros((128, xf.size), np.float64)
    u[0] = (_B0 + 16.0 * 0.6931471805599453) * FEAT_S  # fold 2^16 fp16 scale
    u[1 : KH + 1] = bk * np.cos(ang_x)
    u[KH + 1 : 2 * KH + 1] = bk * np.sin(ang_x)

    ang_y = (2.0 * np.pi) * ks * yf[None, :]
    vv = np.zeros((128, yf.size), np.float64)
    vv[0] = 1.0
    vv[1 : KH + 1] = np.cos(ang_y)
    vv[KH + 1 : 2 * KH + 1] = np.sin(ang_y)

    nb = 2 * KH + 1  # 51 base rows
    split = np.r_[0 : KS + 1, KH + 1 : KH + 1 + KS]  # 37 rows
    ns = split.size

    uh = u.astype(bf16)
    ul = (u - uh.astype(np.float64)).astype(bf16)
    vh = vv.astype(bf16)
    vl = (vv - vh.astype(np.float64)).astype(bf16)

    up = np.zeros((128, xf.size), bf16)
    vp = np.zeros((128, yf.size), bf16)
    up[:nb] = uh[:nb]
    up[nb : nb + ns] = uh[split]
    up[nb + ns : nb + 2 * ns] = ul[split]
    vp[:nb] = vh[:nb]
    vp[nb : nb + ns] = vl[split]
    vp[nb + ns : nb + 2 * ns] = vh[split]
    return up, vp


def _build():
    """Build + compile the per-core Bass/Tile kernel (cached)."""
    global _NC_CACHE
    if _NC_CACHE is not None:
        return _NC_CACHE

    from concourse import bacc, mybir
    import concourse.tile as tile

    f32 = mybir.dt.float32
    f16 = mybir.dt.float16
    bf16 = mybir.dt.bfloat16

    if DVE_TILES:
        exp2q, pow16 = _register_dve_ops()

    nc = bacc.Bacc(
        "TRN2", target_bir_lowering=False, debug=False, num_devices=N_CORES
    )
    ux_d = nc.dram_tensor("ux", [128, MX], bf16, kind="ExternalInput").ap()
    vy_d = nc.dram_tensor("vy", [128, NY], bf16, kind="ExternalInput").ap()
    out_d = nc.dram_tensor("out", [MX, NY], f16, kind="ExternalOutput").ap()

    n_mt = MX // 128   # 8 row blocks
    n_ng = NY // 2048  # 4 vy col groups of 2048
    n_q = NY // 1024   # 8 psum-sized col quarters of 1024 per row block

    with tile.TileContext(nc) as tc:
        with (
            tc.tile_pool(name="wpool", bufs=1) as wpool,
            tc.tile_pool(name="vpool", bufs=n_ng) as vpool,
            tc.tile_pool(name="pspool", bufs=4, space="PSUM") as pspool,
            tc.tile_pool(name="dvepool", bufs=4) as dvepool,
            tc.tile_pool(name="opool", bufs=8) as opool,
        ):
            # input loads.  Each dma_start costs ~0.65 us of SEQUENCER issue
            # time and there are two HWDGE rings (sync + scalar), so the two
            # head-critical pieces (m=0 ux slice, first vy cols) issue from
            # the otherwise-idle Scalar sequencer in parallel with the Sync
            # ring carrying the rest in a few large pieces.
            uxa_t = wpool.tile([128, 128], bf16, name="uxa_t", tag="uxa_t")
            ux_t = wpool.tile([128, MX], bf16, name="ux_t", tag="ux_t")
            vys = []
            for ng in range(n_ng):
                vy_t = vpool.tile([128, 2048], bf16, name=f"vy_{ng}", tag="vy")
                vys.append(vy_t)
            nc.scalar.dma_start(uxa_t[:], ux_d[:, 0:128])
            nc.scalar.dma_start(vys[0][:, 0:1024], vy_d[:, 0:1024])
            nc.sync.dma_start(vys[0][:, 1024:2048], vy_d[:, 1024:2048])
            nc.sync.dma_start(vys[1][:], vy_d[:, 2048:4096])
            # the full ux is only needed from the m=1 row block (~10 us in)
            nc.sync.dma_start(ux_t[:], ux_d[:])
            nc.sync.dma_start(vys[2][:], vy_d[:, 4096:6144])
            nc.sync.dma_start(vys[3][:], vy_d[:, 6144:8192])

            # ACT warm-up: a tiny Exp as soon as the uxa slice lands pulls
            # the ~1.3 us activation table load off the first real drain.
            warm_a = wpool.tile([128, 8], f16, name="warm_a", tag="warm_a")
            nc.scalar.activation(
                warm_a[:],
                uxa_t[:, 0:8],
                mybir.ActivationFunctionType.Exp,
                scale=0.0,
            )

            # PE warm-up: dummy matmuls on the early uxa tile keep the PE
            # busy while the vy groups stream in, so the HAM clock gate is
            # at 2.4 GHz when the real matmuls start.
            warm_ps = pspool.tile([128, 128], f32, name="warm_ps", tag="ps")
            for _w in range(14):
                nc.tensor.matmul(
                    warm_ps[:],
                    uxa_t[:],
                    uxa_t[:],
                    start=True,
                    stop=True,
                )

            for m in range(n_mt):
                msl = slice(m * 128, (m + 1) * 128)
                ux_src = uxa_t[:, 0:128] if m == 0 else ux_t[:, msl]
                for cg in range(n_q // 2):  # 2048-col output chunks
                    out_t = opool.tile(
                        [128, 2048], f16, name=f"out_{m}_{cg}", tag="out_t"
                    )
                    for sub in range(2):
                        q = cg * 2 + sub
                        osl = slice(sub * 1024, (sub + 1) * 1024)
                        ps = pspool.tile(
                            [128, 1024], f32, name=f"ps_{m}_{q}", tag="ps"
                        )
                        for s in range(2):
                            ssl = slice(s * 512, (s + 1) * 512)
                            nc.tensor.matmul(
                                ps[:, ssl],
                                ux_src,
                                vys[q // 2][:, (q % 2) * 1024 + s * 512 :
                                            (q % 2) * 1024 + (s + 1) * 512],
                                start=True, stop=True,
                            )
                        if (m * n_q + q) in DVE_TILES:
                            # DVE drain: monic quartic 2^v then w^16
                            w_t = dvepool.tile(
                                [128, 1024], f32, name=f"w_{m}_{q}", tag="w_t"
                            )
                            nc.vector._custom_dve(
                                exp2q, out=w_t[:], in0=ps[:],
                                s0=QA, s1=QB, imm2=QC,
                            )
                            nc.vector._custom_dve(
                                pow16, out=out_t[:, osl], in0=w_t[:],
                            )
                        else:
                            # ACT drain: fp16 out = exp(v * 16 lam ln2)
                            # = K * 2^16, always in fp16 normal range
                            nc.scalar.activation(
                                out_t[:, osl],
                                ps[:],
                                mybir.ActivationFunctionType.Exp,
                                scale=ACT_SCALE,
                            )
                    nc.sync.dma_start(
                        out_d[msl, cg * 2048 : (cg + 1) * 2048],
                        out_t[:],
                    )

    nc.compile()
    _NC_CACHE = nc
    return nc


def kernel(x: np.ndarray, y: np.ndarray) -> np.ndarray:
    global LAST_EXEC_TIME_NS, LAST_TRACE_PATH
    from concourse import bass_utils

    up, vp = _features(x, y)
    nc = _build()

    in_maps = [
        {"ux": np.ascontiguousarray(up[:, i * MX : (i + 1) * MX]), "vy": vp}
        for i in range(N_CORES)
    ]
    trace = bool(os.environ.get("BESSEL_TRACE"))
    res = bass_utils.run_bass_kernel_spmd(
        nc, in_maps, core_ids=list(range(N_CORES)), trace=trace
    )
    LAST_EXEC_TIME_NS = res.exec_time_ns
    if res.instructions_and_trace is not None:
        LAST_TRACE_PATH = res.instructions_and_trace[1]
    out = np.empty((NX, NY), np.float32)
    for i in range(N_CORES):
        blk = out[i * MX : (i + 1) * MX]
        np.multiply(
            res.results[i]["out"].astype(np.float32),
            np.float32(2.0**-16),
            out=blk,
        )
    return out
